# revision 1
# baseline (speedup 1.0000x reference)
"""Trainium2 Bass kernel for nn_AwareDecoder segment first/last gather.

Problem: input [16, 2048, 1024] f32, number_mask [16, 2048] int64 with ids in
[0, 512]. For each segment id i in [0, 512): find first/last row-major token
position with that id, gather those rows of the flattened input, concat ->
out [512, 2048] f32.

Strategy (8 NeuronCores, segment-sharded - no collectives):
  core c owns segments [64c, 64c+64). Each core:
    - DMAs the (tiny, 256KB) id array, extracts int64 low words,
    - computes per-segment min/max token position with an fp16 eq/select/
      reduce sweep on the vector engine. Token chunks sit on partitions and
      positions are encoded chunk-LOCALLY (values <= 256, fp16-exact) so the
      four mult/reduce passes run in the DVE 2x packed mode; the global
      position is reconstructed in the tiny post-transpose stage,
    - PE-transpose + free-axis reduce for the cross-partition combine,
    - gathers its 64 first + 64 last rows (4KB each) straight from HBM with
      one hardware indirect DMA (reads only 512KB of the 128MB input),
    - writes its [64, 2048] slice of the output.
Host concatenates the 8 slices.
"""
import numpy as np

import concourse.bass as bass
import concourse.tile as tile
from concourse import bacc, mybir
from concourse import bass_utils
from concourse.masks import make_identity

P = 128            # partitions
L = 32768          # B*S tokens
H = 1024           # hidden
NSEG = 512         # segments
NCORES = 8
SEG_PER_CORE = NSEG // NCORES            # 64
TOK_PER_PART = L // P                    # 256 tokens per partition
F32 = mybir.dt.float32
F16 = mybir.dt.float16
I32 = mybir.dt.int32


def build_nc():
    nc = bacc.Bacc("TRN2", target_bir_lowering=False, debug=False)

    x = nc.dram_tensor("x", [L, H], F32, kind="ExternalInput")
    # number_mask int64 raw bytes as int32 (lo, hi) pairs; partition p covers
    # tokens [p*256, (p+1)*256).
    idpairs = nc.dram_tensor("idpairs", [P, TOK_PER_PART, 2], I32, kind="ExternalInput")
    # packed fp16 consts (per-core): [c8hi (8*256) | c8lo (8*256) | posmin | posmax]
    cpack_in = nc.dram_tensor("cpack", [P, 18 * TOK_PER_PART], F16,
                              kind="ExternalInput")
    # global-position bases for the post-transpose decode:
    # rows 0..63   (min side): base[s, p] = (127 - p) * 256
    # rows 64..127 (max side): base[s, p] = p * 256
    base_in = nc.dram_tensor("base", [2, SEG_PER_CORE, P], F32, kind="ExternalInput")
    out = nc.dram_tensor("out", [SEG_PER_CORE, 2 * H], F32, kind="ExternalOutput")

    with tile.TileContext(nc) as tc:
        with tc.tile_pool(name="sb", bufs=1) as sb, \
             tc.tile_pool(name="big", bufs=1) as big, \
             tc.tile_pool(name="ps", bufs=1, space="PSUM") as ps:

            # ---- load ids, extract low int32 words, cast to fp16 ----
            idp_t = sb.tile([P, TOK_PER_PART, 2], I32)
            nc.sync.dma_start(idp_t[:], idpairs.ap())
            cpack = sb.tile([P, 18 * TOK_PER_PART], F16)
            nc.scalar.dma_start(cpack[:], cpack_in.ap())
            c8hi_t = cpack[:, 0:8 * TOK_PER_PART].rearrange(
                "p (a t) -> p a t", a=8)
            c8lo_t = cpack[:, 8 * TOK_PER_PART:16 * TOK_PER_PART].rearrange(
                "p (a t) -> p a t", a=8)
            posmin = cpack[:, 16 * TOK_PER_PART:17 * TOK_PER_PART]
            posmax = cpack[:, 17 * TOK_PER_PART:18 * TOK_PER_PART]
            base_t = sb.tile([P, P], F32)
            nc.gpsimd.dma_start(base_t[:], base_in.ap().rearrange("a s p -> (a s) p"))

            # ---- factorized seg compare: id>>3 == base/8 + m, id&7 == lo ----
            hi_i = sb.tile([P, TOK_PER_PART], I32)
            nc.vector.tensor_scalar(hi_i[:], idp_t[:, :, 0], 3, None,
                                    op0=mybir.AluOpType.arith_shift_right)
            lo_i = sb.tile([P, TOK_PER_PART], I32)
            nc.vector.tensor_scalar(lo_i[:], idp_t[:, :, 0], 7, None,
                                    op0=mybir.AluOpType.bitwise_and)
            hi_f = sb.tile([P, TOK_PER_PART], F16)
            nc.vector.tensor_copy(hi_f[:], hi_i[:])
            lo_f = sb.tile([P, TOK_PER_PART], F16)
            nc.vector.tensor_copy(lo_f[:], lo_i[:])

            eq_hi = sb.tile([P, 8, TOK_PER_PART], F16)
            nc.vector.tensor_tensor(
                out=eq_hi[:],
                in0=hi_f[:].unsqueeze(1).broadcast_to([P, 8, TOK_PER_PART]),
                in1=c8hi_t, op=mybir.AluOpType.is_equal)
            eq_lo = sb.tile([P, 8, TOK_PER_PART], F16)
            nc.vector.tensor_tensor(
                out=eq_lo[:],
                in0=lo_f[:].unsqueeze(1).broadcast_to([P, 8, TOK_PER_PART]),
                in1=c8lo_t, op=mybir.AluOpType.is_equal)
            eqlo_min = sb.tile([P, 8, TOK_PER_PART], F16)
            nc.vector.tensor_tensor(
                out=eqlo_min[:], in0=eq_lo[:],
                in1=posmin.unsqueeze(1).broadcast_to([P, 8, TOK_PER_PART]),
                op=mybir.AluOpType.mult)
            eqlo_max = sb.tile([P, 8, TOK_PER_PART], F16)
            nc.vector.tensor_tensor(
                out=eqlo_max[:], in0=eq_lo[:],
                in1=posmax.unsqueeze(1).broadcast_to([P, 8, TOK_PER_PART]),
                op=mybir.AluOpType.mult)

            # ---- big fused candidate passes (2x) + reduces ----
            cand = big.tile([P, 8, 8, TOK_PER_PART], F16)
            nc.vector.tensor_tensor(
                out=cand[:],
                in0=eq_hi[:].unsqueeze(2).broadcast_to([P, 8, 8, TOK_PER_PART]),
                in1=eqlo_min[:].unsqueeze(1).broadcast_to([P, 8, 8, TOK_PER_PART]),
                op=mybir.AluOpType.mult)
            # TT-max tree (2x) then small reduce: 256 -> 32 -> 1
            red = sb.tile([P, P], F16)  # [:, :64] min-enc, [:, 64:] max-enc
            c3 = cand[:].rearrange("p a b t -> p (a b) t")
            lv1 = big.tile([P, SEG_PER_CORE, 128], F16, tag="lv1")
            nc.vector.tensor_tensor(out=lv1[:], in0=c3[:, :, 0:128],
                                    in1=c3[:, :, 128:256], op=mybir.AluOpType.max)
            lv2 = sb.tile([P, SEG_PER_CORE, 64], F16, tag="lv2")
            nc.vector.tensor_tensor(out=lv2[:], in0=lv1[:, :, 0:64],
                                    in1=lv1[:, :, 64:128], op=mybir.AluOpType.max)
            lv3 = sb.tile([P, SEG_PER_CORE, 32], F16, tag="lv3")
            nc.vector.tensor_tensor(out=lv3[:], in0=lv2[:, :, 0:32],
                                    in1=lv2[:, :, 32:64], op=mybir.AluOpType.max)
            nc.vector.tensor_reduce(red[:, 0:SEG_PER_CORE], lv3[:],
                                    axis=mybir.AxisListType.X,
                                    op=mybir.AluOpType.max)
            cand2 = big.tile([P, 8, 8, TOK_PER_PART], F16)
            nc.vector.tensor_tensor(
                out=cand2[:],
                in0=eq_hi[:].unsqueeze(2).broadcast_to([P, 8, 8, TOK_PER_PART]),
                in1=eqlo_max[:].unsqueeze(1).broadcast_to([P, 8, 8, TOK_PER_PART]),
                op=mybir.AluOpType.mult)
            c3b = cand2[:].rearrange("p a b t -> p (a b) t")
            lv1b = big.tile([P, SEG_PER_CORE, 128], F16, tag="lv1")
            nc.vector.tensor_tensor(out=lv1b[:], in0=c3b[:, :, 0:128],
                                    in1=c3b[:, :, 128:256], op=mybir.AluOpType.max)
            lv2b = sb.tile([P, SEG_PER_CORE, 64], F16, tag="lv2")
            nc.vector.tensor_tensor(out=lv2b[:], in0=lv1b[:, :, 0:64],
                                    in1=lv1b[:, :, 64:128], op=mybir.AluOpType.max)
            lv3b = sb.tile([P, SEG_PER_CORE, 32], F16, tag="lv3")
            nc.vector.tensor_tensor(out=lv3b[:], in0=lv2b[:, :, 0:32],
                                    in1=lv2b[:, :, 32:64], op=mybir.AluOpType.max)
            nc.vector.tensor_reduce(red[:, SEG_PER_CORE:P], lv3b[:],
                                    axis=mybir.AxisListType.X,
                                    op=mybir.AluOpType.max)

            # ---- cross-partition combine, decode, gather ----
            ident = sb.tile([P, P], F16)
            make_identity(nc, ident[:])
            red_t = ps.tile([P, P], F16)
            nc.tensor.transpose(out=red_t[:], in_=red[:], identity=ident[:])
            mask = sb.tile([P, P], F32)
            nc.vector.tensor_scalar(mask[:], red_t[:], 0.0, None,
                                    op0=mybir.AluOpType.is_gt)
            glob = sb.tile([P, P], F32)
            nc.vector.tensor_tensor(out=glob[:], in0=red_t[:], in1=base_t[:],
                                    op=mybir.AluOpType.add)
            nc.vector.tensor_tensor(out=glob[:], in0=glob[:], in1=mask[:],
                                    op=mybir.AluOpType.mult)
            enc = sb.tile([P, 1], F32)
            nc.vector.tensor_reduce(enc[:], glob[:],
                                    axis=mybir.AxisListType.X,
                                    op=mybir.AluOpType.max)
            idx_f = sb.tile([P, 1], F32)
            nc.vector.tensor_scalar(idx_f[0:SEG_PER_CORE, :], enc[0:SEG_PER_CORE, :],
                                    -1.0, float(L),
                                    op0=mybir.AluOpType.mult,
                                    op1=mybir.AluOpType.add)
            nc.vector.tensor_scalar_add(idx_f[SEG_PER_CORE:P, :],
                                        enc[SEG_PER_CORE:P, :], -1.0)
            idx_i = sb.tile([P, 1], I32)
            nc.vector.tensor_copy(idx_i[:], idx_f[:])
            rows = big.tile([P, H], F32)
            nc.gpsimd.indirect_dma_start(
                out=rows[:], out_offset=None, in_=x.ap(),
                in_offset=bass.IndirectOffsetOnAxis(ap=idx_i[:, 0:1], axis=0))
            nc.gpsimd.dma_start(out.ap()[:, 0:H], rows[0:SEG_PER_CORE, :])
            nc.sync.dma_start(out.ap()[:, H:2 * H], rows[SEG_PER_CORE:P, :])

    nc.compile()
    return nc


_NC = None


def _get_nc():
    global _NC
    if _NC is None:
        _NC = build_nc()
    return _NC


def make_in_maps(input, number_mask):
    x = np.ascontiguousarray(np.asarray(input), dtype=np.float32).reshape(L, H)
    nm = np.ascontiguousarray(np.asarray(number_mask))
    if nm.dtype != np.int64:
        nm = nm.astype(np.int64)
    idpairs = nm.reshape(L).view(np.int32).reshape(P, TOK_PER_PART, 2)
    c8lo = np.repeat(np.arange(8, dtype=np.float16), TOK_PER_PART)
    f = np.arange(TOK_PER_PART, dtype=np.float16)
    pcol = np.arange(P, dtype=np.float32)
    base = np.empty((2, SEG_PER_CORE, P), dtype=np.float32)
    base[0] = (P - 1 - pcol) * TOK_PER_PART
    base[1] = pcol * TOK_PER_PART
    in_maps = []
    for c in range(NCORES):
        c8hi = np.repeat(np.arange(8, dtype=np.float16) + c * 8, TOK_PER_PART)
        cpack = np.tile(np.concatenate([c8hi, c8lo, TOK_PER_PART - f, f + 1]),
                        (P, 1))
        in_maps.append({"x": x, "idpairs": idpairs, "cpack": cpack,
                        "base": base})
    return in_maps


def kernel(input, number_mask, n, concat, **_):
    assert int(n) == NSEG and int(concat) == 1
    nc = _get_nc()
    in_maps = make_in_maps(input, number_mask)
    res = bass_utils.run_bass_kernel_spmd(nc, in_maps, core_ids=list(range(NCORES)))
    return np.concatenate([res.results[c]["out"] for c in range(NCORES)], axis=0)



# revision 6
# speedup vs baseline: 1.0763x; 1.0763x over previous
"""Trainium2 Bass kernel for nn_AwareDecoder segment first/last gather.

Problem: input [16, 2048, 1024] f32, number_mask [16, 2048] int64 with ids in
[0, 512]. For each segment id i in [0, 512): find first/last row-major token
position with that id, gather those rows of the flattened input, concat ->
out [512, 2048] f32.

Strategy (8 NeuronCores, segment-sharded, no collectives). Core c owns
segments [64c, 64c+64). Token t = 128*g + q with q on partitions, chunk
g in [0, 256); g = 16*w + r (window w in [0,16), class r in [0,16)).

  1. DVE: one double-broadcast is_equal builds the 0/1 mask
     [128q, 64s, 256g] bf16 (2x packed mode), in 4 g-chunks.
  2. PE: matmuls against fixed stationaries W_c[q, 2r+j] = 2^(q-64) (j=0,
     last) / 2^(63-q) (j=1, first) for r==c else 0, PSUM-accumulated over
     c. A sum of distinct powers of 2 per (s, w, r): the f32 exponent IS
     the winning q (first/last occurrence within the 128-token chunk).
  3. Act drains PSUM [32, (w s)] -> SBUF; 16 PE transposes put segments
     on partitions: [64s, 16w, 32(r,j)] f32.
  4. DVE decode: exponent extract via bitcast>>23, add a position table,
     kill empty chunks, max-reduce -> first/last token index per segment.
  5. Two hardware indirect DMAs gather the 64+64 rows (4KB each) from
     HBM; two direct DMAs write the [64, 2048] output slice.
Host concatenates the 8 slices.
"""
import numpy as np
import ml_dtypes

import concourse.bass as bass
import concourse.tile as tile
from concourse import bacc, mybir
from concourse import bass_utils
from concourse.masks import make_identity

P = 128            # partitions / tokens per chunk
L = 32768          # B*S tokens
H = 1024           # hidden
NSEG = 512         # segments
NCORES = 8
SEG_PER_CORE = NSEG // NCORES            # 64
NG = L // P                              # 256 chunks
NW = 16                                  # windows
NR = NG // NW                            # 16 chunk classes per window
F32 = mybir.dt.float32
F16 = mybir.dt.float16
BF16 = mybir.dt.bfloat16
I32 = mybir.dt.int32

NCHUNK = 4                 # mask/matmul pipeline chunks (4 windows each)
WPC = NW // NCHUNK         # windows per chunk


def build_nc():
    nc = bacc.Bacc("TRN2", target_bir_lowering=False, debug=False)

    x = nc.dram_tensor("x", [L, H], F32, kind="ExternalInput")
    # ids_mat[q, g] = number_mask[128*g + q] as f16 (ids <= 512, f16-exact)
    ids_in = nc.dram_tensor("ids", [P, NG], F16, kind="ExternalInput")
    # segc[q, 2s+e] = 64*core + s (duplicated pairs for packed innermost dim)
    segc_in = nc.dram_tensor("segc", [P, 2 * SEG_PER_CORE], F16,
                             kind="ExternalInput")
    # 16 stationaries W_c[q, 2r+j], c-major
    w_in = nc.dram_tensor("wstack", [P, NR * 2 * NR], BF16, kind="ExternalInput")
    # decode table t1[s, 32w + 2r + j]
    t1_in = nc.dram_tensor("t1", [SEG_PER_CORE, NW * 2 * NR], I32,
                           kind="ExternalInput")
    out = nc.dram_tensor("out", [SEG_PER_CORE, 2 * H], F32, kind="ExternalOutput")

    with tile.TileContext(nc) as tc:
        with tc.tile_pool(name="sb", bufs=1) as sb, \
             tc.tile_pool(name="big", bufs=1) as big, \
             tc.tile_pool(name="ps", bufs=1, space="PSUM") as ps:

            ids_t = sb.tile([P, NG], F16)
            nc.sync.dma_start(ids_t[:], ids_in.ap())
            segc_t = sb.tile([P, 2 * SEG_PER_CORE], F16)
            nc.scalar.dma_start(segc_t[:], segc_in.ap())
            w_t = sb.tile([P, NR * 2 * NR], BF16)
            nc.gpsimd.dma_start(w_t[:], w_in.ap())
            t1_t = sb.tile([SEG_PER_CORE, NW * 2 * NR], I32)
            nc.scalar.dma_start(t1_t[:], t1_in.ap())
            ident = sb.tile([32, 32], F32)
            make_identity(nc, ident[:])

            # ---- mask[q, s, g] = (id[q, g] == 64c + s), bf16 0/1 ----
            mask = big.tile([P, SEG_PER_CORE, NG], BF16)
            seg_bcast = segc_t[:].rearrange("p (s e) -> p s e", e=2) \
                .unsqueeze(2)
            ps_mm = ps.tile([2 * NR, NW * SEG_PER_CORE], F32)
            drained = sb.tile([2 * NR, NW * SEG_PER_CORE], F32)
            gpw = NG // NCHUNK            # chunks of g per pipeline step
            m4 = mask[:].rearrange("p s (w r) -> p w s r", r=NR)
            for k in range(NCHUNK):
                g0 = k * gpw
                nc.vector.tensor_tensor(
                    out=mask[:, :, g0:g0 + gpw]
                        .rearrange("p s (x e) -> p s x e", e=2),
                    in0=ids_t[:, g0:g0 + gpw]
                        .rearrange("p (x e) -> p x e", e=2).unsqueeze(1)
                        .broadcast_to([P, SEG_PER_CORE, gpw // 2, 2]),
                    in1=seg_bcast.broadcast_to([P, SEG_PER_CORE, gpw // 2, 2]),
                    op=mybir.AluOpType.is_equal)
                # matmuls for windows in this chunk: cols (w, s)
                for c in range(NR):
                    nc.tensor.matmul(
                        out=ps_mm[:, k * WPC * SEG_PER_CORE:
                                  (k + 1) * WPC * SEG_PER_CORE],
                        lhsT=w_t[:, 2 * NR * c:2 * NR * (c + 1)],
                        rhs=m4[:, k * WPC:(k + 1) * WPC, :, c],
                        start=(c == 0), stop=(c == NR - 1))
                nc.scalar.activation(
                    drained[:, k * WPC * SEG_PER_CORE:
                            (k + 1) * WPC * SEG_PER_CORE],
                    ps_mm[:, k * WPC * SEG_PER_CORE:
                          (k + 1) * WPC * SEG_PER_CORE],
                    mybir.ActivationFunctionType.Copy)

            # ---- transpose: per window w, [32(r,j), 64s] -> [64s, 32] ----
            tps = ps.tile([SEG_PER_CORE, NW, 2 * NR], F32)
            for w in range(NW):
                nc.tensor.transpose(
                    out=tps[:, w, :],
                    in_=drained[:, w * SEG_PER_CORE:(w + 1) * SEG_PER_CORE],
                    identity=ident[:])

            # ---- decode: val = (exp_bits + t1) * (E > 0); max-reduce ----
            FREE = NW * 2 * NR                         # 512
            ebs = sb.tile([SEG_PER_CORE, FREE], I32)
            nc.vector.tensor_scalar(ebs[:], tps[:].rearrange("p w m -> p (w m)")
                                    .bitcast(I32), 23, None,
                                    op0=mybir.AluOpType.logical_shift_right)
            ind = sb.tile([SEG_PER_CORE, FREE], I32)
            nc.gpsimd.tensor_scalar(ind[:], ebs[:], 0, None,
                                    op0=mybir.AluOpType.is_gt)
            val = sb.tile([SEG_PER_CORE, FREE], I32)
            nc.vector.tensor_tensor(out=val[:], in0=ebs[:], in1=t1_t[:],
                                    op=mybir.AluOpType.add)
            nc.vector.tensor_tensor(out=val[:], in0=val[:], in1=ind[:],
                                    op=mybir.AluOpType.mult)
            red = sb.tile([SEG_PER_CORE, 2], I32)
            nc.vector.tensor_reduce(red[:],
                                    val[:].rearrange("p (u j) -> p j u", j=2),
                                    axis=mybir.AxisListType.X,
                                    op=mybir.AluOpType.max)
            # idx_last = red[:,0] - 1 ; idx_first = 32769 - red[:,1]
            idx = sb.tile([SEG_PER_CORE, 2], I32)
            nc.vector.tensor_scalar_add(idx[:, 0:1], red[:, 0:1], -1)
            nc.vector.tensor_scalar(idx[:, 1:2], red[:, 1:2], -1, 32769,
                                    op0=mybir.AluOpType.mult,
                                    op1=mybir.AluOpType.add)

            # ---- gather rows + write out ----
            rows_f = big.tile([SEG_PER_CORE, H], F32)
            rows_l = big.tile([SEG_PER_CORE, H], F32)
            nc.gpsimd.indirect_dma_start(
                out=rows_f[:], out_offset=None, in_=x.ap(),
                in_offset=bass.IndirectOffsetOnAxis(ap=idx[:, 1:2], axis=0))
            nc.gpsimd.indirect_dma_start(
                out=rows_l[:], out_offset=None, in_=x.ap(),
                in_offset=bass.IndirectOffsetOnAxis(ap=idx[:, 0:1], axis=0))
            nc.scalar.dma_start(out.ap()[:, 0:H], rows_f[:])
            nc.sync.dma_start(out.ap()[:, H:2 * H], rows_l[:])

    nc.compile()
    return nc


_NC = None


def _get_nc():
    global _NC
    if _NC is None:
        _NC = build_nc()
    return _NC


def make_in_maps(input, number_mask):
    x = np.ascontiguousarray(np.asarray(input), dtype=np.float32).reshape(L, H)
    nm = np.asarray(number_mask).reshape(L).astype(np.int64)
    ids = np.ascontiguousarray(nm.reshape(NG, P).T).astype(np.float16)

    q = np.arange(P, dtype=np.int64)
    wstack = np.zeros((P, NR, 2 * NR), dtype=np.float64)
    for c in range(NR):
        wstack[:, c, 2 * c] = np.exp2(q - 64.0)
        wstack[:, c, 2 * c + 1] = np.exp2(63.0 - q)
    wstack = wstack.reshape(P, NR * 2 * NR).astype(ml_dtypes.bfloat16)

    w_i, r_i = np.meshgrid(np.arange(NW), np.arange(NR), indexing="ij")
    g = (NR * w_i + r_i).astype(np.int64)          # [NW, NR]
    t1 = np.empty((NW, NR, 2), dtype=np.int32)
    t1[:, :, 0] = 128 * g - 62
    t1[:, :, 1] = 32579 - 128 * g
    t1 = np.tile(t1.reshape(1, NW * 2 * NR), (SEG_PER_CORE, 1))

    in_maps = []
    for c in range(NCORES):
        segc = np.repeat(np.arange(SEG_PER_CORE, dtype=np.float16)
                         + c * SEG_PER_CORE, 2)
        in_maps.append({"x": x, "ids": ids,
                        "segc": np.tile(segc, (P, 1)),
                        "wstack": wstack, "t1": t1})
    return in_maps


def kernel(input, number_mask, n, concat, **_):
    assert int(n) == NSEG and int(concat) == 1
    nc = _get_nc()
    in_maps = make_in_maps(input, number_mask)
    res = bass_utils.run_bass_kernel_spmd(nc, in_maps, core_ids=list(range(NCORES)))
    return np.concatenate([res.results[c]["out"] for c in range(NCORES)], axis=0)


# revision 17
# speedup vs baseline: 1.6555x; 1.5382x over previous
"""Trainium2 Bass kernel for nn_AwareDecoder segment first/last gather.

Problem: input [16, 2048, 1024] f32, number_mask [16, 2048] int64 with ids in
[0, 512]. For each segment id i in [0, 512): find first/last row-major token
position with that id, gather those rows of the flattened input, concat ->
out [512, 2048] f32.

Strategy (8 NeuronCores, segment-sharded, no collectives). Core c owns
segments [64c, 64c+64). Token t = 128*g + q with q on partitions, chunk
g in [0, 256); g = 8*cls + gl (class cls in [0, 32), offset gl in [0, 8)).

  1. DVE: one double-broadcast is_equal builds the 0/1 mask
     [128q, 256g, 64s] bf16 (2x packed mode), in 4 g-chunks.
  2. PE: one 512-col matmul per class cls against a fixed stationary
     W[q, 2r+j] = 2^(q-64) (j=0, last) / 2^(63-q) (j=1, first), PSUM-
     accumulated over 16 classes per half. A sum of distinct powers of 2
     per (s, g): the f32 exponent IS the winning q within the chunk.
  3. Act drains PSUM -> SBUF bf16; DMA-transposes (XBAR) put (gl parity,
     seg) on partitions with zero engine time.
  4. DVE int16 decode (4x/2x modes): exponent extract via bitcast>>7,
     add a position table, kill empty chunks, max-reduce, combine halves
     and parities -> first/last token index per segment.
  5. One hardware indirect DMA gathers the 128 rows (4KB each) from HBM;
     direct DMAs write the [64, 2048] output slice.
Host concatenates the 8 slices.
"""
import numpy as np
import ml_dtypes

import concourse.bass as bass
import concourse.tile as tile
from concourse import bacc, mybir
from concourse import bass_utils

P = 128            # partitions / tokens per chunk
L = 32768          # B*S tokens
H = 1024           # hidden
NSEG = 512         # segments
NCORES = 8
SEG = NSEG // NCORES                     # 64 segments per core
NG = L // P                              # 256 chunks
GL = 8                                   # chunk offsets per class
NCLS = NG // GL                          # 32 classes
F32 = mybir.dt.float32
F16 = mybir.dt.float16
BF16 = mybir.dt.bfloat16
I16 = mybir.dt.int16
I32 = mybir.dt.int32

NCHUNK = 4                 # mask/matmul pipeline chunks
GPC = NG // NCHUNK         # 64 g per chunk
CPC = NCLS // NCHUNK       # 8 classes per chunk


def build_nc():
    nc = bacc.Bacc("TRN2", target_bir_lowering=False, debug=False)

    x = nc.dram_tensor("x", [L, H], F32, kind="ExternalInput")
    # fpack[q, :2*NG] = ids duplicated pairs; fpack[q, 2*NG:] = segp (f16)
    fpack_in = nc.dram_tensor("fpack", [P, 2 * NG + SEG], F16,
                              kind="ExternalInput")
    # 32 stationaries W_c[q, 2r'+j], c-major
    w_in = nc.dram_tensor("wstack", [P, NCLS * 32], BF16, kind="ExternalInput")
    # decode table t1[(parity, s), 128h + 32q + 2r' + j]
    t1_in = nc.dram_tensor("t1", [P, 256], I16, kind="ExternalInput")
    out = nc.dram_tensor("out", [SEG, 2 * H], F32, kind="ExternalOutput")

    with tile.TileContext(nc) as tc:
        with tc.tile_pool(name="sb", bufs=1) as sb, \
             tc.tile_pool(name="big", bufs=1) as big, \
             tc.tile_pool(name="ps", bufs=1, space="PSUM") as ps:

            fpack_t = sb.tile([P, 2 * NG + SEG], F16)
            nc.sync.dma_start(fpack_t[:], fpack_in.ap())
            ids_t = fpack_t[:, 0:2 * NG]
            segp_t = fpack_t[:, 2 * NG:]
            w_t = sb.tile([P, NCLS * 32], BF16)
            nc.gpsimd.dma_start(w_t[:], w_in.ap())
            t1_t = sb.tile([P, 256], I16)
            nc.gpsimd.dma_start(t1_t[:], t1_in.ap())

            # ---- mask[q, g, s] = (id[q, g] == 64c + s), bf16 0/1 ----
            mask = big.tile([P, NG, SEG], BF16)
            ps_mm = ps.tile([2 * NCLS // 2, 512], F32)      # [32, 512] x2 halves
            ps_mm2 = ps.tile([32, 512], F32)
            drained0 = sb.tile([32, 512], BF16)
            drained1 = sb.tile([32, 512], BF16)
            drained = [drained0, drained1]
            tps = sb.tile([P, 2, 4, 32], BF16)
            psh = [ps_mm, ps_mm2]

            for k in range(NCHUNK):
                g0 = k * GPC
                nc.vector.tensor_tensor(
                    out=mask[:, g0:g0 + GPC, :]
                        .rearrange("p g (a b) -> p g a b", b=2),
                    in0=ids_t[:, 2 * g0:2 * (g0 + GPC)]
                        .rearrange("p (g e) -> p g e", e=2).unsqueeze(2)
                        .broadcast_to([P, GPC, SEG // 2, 2]),
                    in1=segp_t.rearrange("p (a b) -> p a b", b=2)
                        .unsqueeze(1).broadcast_to([P, GPC, SEG // 2, 2]),
                    op=mybir.AluOpType.is_equal)
                for cc in range(CPC):
                    c = k * CPC + cc
                    h, r = c // 16, c % 16
                    nc.tensor.matmul(
                        out=psh[h][:, :],
                        lhsT=w_t[:, 32 * c:32 * (c + 1)],
                        rhs=mask[:, GL * c:GL * (c + 1), :],
                        start=(r == 0), stop=(r == 15))
                if k % 2 == 1:                    # half h = k//2 complete
                    h = k // 2
                    nc.scalar.activation(drained[h][:], psh[h][:],
                                         mybir.ActivationFunctionType.Copy)
                    for q in range(4):
                        eng = nc.sync if q % 2 == 0 else nc.scalar
                        eng.dma_start(tps[:, h, q, :],
                                      drained[h][:, 128 * q:128 * (q + 1)],
                                      transpose=True)

            # ---- int16 decode ----
            red = sb.tile([P, 2, 2], I16)
            for h in range(2):
                src = tps[:, h, :, :].rearrange("p q m -> p (q m)")
                ebs = sb.tile([P, 128], I16, tag="ebs")
                nc.vector.tensor_scalar(ebs[:], src.bitcast(I16), 7, None,
                                        op0=mybir.AluOpType.logical_shift_right)
                ind = sb.tile([P, 128], I16, tag="ind")
                nc.vector.tensor_scalar(ind[:], ebs[:], 0, None,
                                        op0=mybir.AluOpType.is_gt)
                val = sb.tile([P, 128], I16, tag="val")
                nc.vector.tensor_tensor(out=val[:], in0=ebs[:],
                                        in1=t1_t[:, 128 * h:128 * (h + 1)],
                                        op=mybir.AluOpType.add)
                nc.vector.tensor_tensor(out=val[:], in0=val[:], in1=ind[:],
                                        op=mybir.AluOpType.mult)
                nc.vector.tensor_reduce(red[:, h, :],
                                        val[:].rearrange("p (u j) -> p j u", j=2),
                                        axis=mybir.AxisListType.X,
                                        op=mybir.AluOpType.max)
            redc = sb.tile([P, 2], I16)
            nc.vector.tensor_tensor(out=redc[:], in0=red[:, 0, :],
                                    in1=red[:, 1, :], op=mybir.AluOpType.max)
            # combine gl-parities (partitions p and p+64) via tiny DMA realign
            redhi = sb.tile([SEG, 2], I16)
            nc.sync.dma_start(redhi[:], redc[SEG:P, :])
            fin = sb.tile([SEG, 2], I16)
            nc.vector.tensor_tensor(out=fin[:], in0=redc[0:SEG, :],
                                    in1=redhi[:], op=mybir.AluOpType.max)
            idx = sb.tile([SEG, 2], I32)
            nc.vector.tensor_scalar(idx[:, 0:1], fin[:, 1:2], -1, 32767,
                                    op0=mybir.AluOpType.mult,
                                    op1=mybir.AluOpType.add)
            nc.vector.tensor_scalar_add(idx[:, 1:2], fin[:, 0:1], 0)

            # ---- gather rows + write out ----
            rows_f = big.tile([SEG, H], F32)
            rows_l = big.tile([SEG, H], F32)
            nc.gpsimd.indirect_dma_start(
                out=rows_f[:], out_offset=None, in_=x.ap(),
                in_offset=bass.IndirectOffsetOnAxis(ap=idx[:, 0:1], axis=0),
                bounds_check=L - 1, oob_is_err=False)
            nc.gpsimd.indirect_dma_start(
                out=rows_l[:], out_offset=None, in_=x.ap(),
                in_offset=bass.IndirectOffsetOnAxis(ap=idx[:, 1:2], axis=0),
                bounds_check=L - 1, oob_is_err=False)
            nc.scalar.dma_start(out.ap()[:, 0:H], rows_f[:])
            nc.sync.dma_start(out.ap()[:, H:2 * H], rows_l[:])

    nc.compile()
    return nc


_NC = None


def _get_nc():
    global _NC
    if _NC is None:
        _NC = build_nc()
    return _NC


def make_in_maps(input, number_mask):
    x = np.ascontiguousarray(np.asarray(input), dtype=np.float32).reshape(L, H)
    nm = np.asarray(number_mask).reshape(L).astype(np.int64)
    ids = np.ascontiguousarray(nm.reshape(NG, P).T).astype(np.float16)
    ids2 = np.repeat(ids, 2, axis=1)

    q = np.arange(P, dtype=np.float64)
    wstack = np.zeros((P, NCLS, 32), dtype=np.float64)
    for c in range(NCLS):
        r = c % 16
        wstack[:, c, 2 * r] = np.exp2(q - 64.0)
        wstack[:, c, 2 * r + 1] = np.exp2(63.0 - q)
    wstack = wstack.reshape(P, NCLS * 32).astype(ml_dtypes.bfloat16)

    t1 = np.empty((P, 2, 4, 16, 2), dtype=np.int16)
    p = np.arange(P)
    parity = (p >> 6) & 1                                  # [P]
    for h in range(2):
        for qd in range(4):
            for r in range(16):
                g = 8 * (16 * h + r) + 2 * qd + parity     # [P]
                t1[:, h, qd, r, 0] = (128 * g - 63).astype(np.int16)
                t1[:, h, qd, r, 1] = (32577 - 128 * g).astype(np.int16)
    t1 = t1.reshape(P, 256)

    in_maps = []
    for c in range(NCORES):
        segp = np.tile(np.arange(SEG, dtype=np.float16) + c * SEG, (P, 1))
        in_maps.append({"x": x,
                        "fpack": np.concatenate([ids2, segp], axis=1),
                        "wstack": wstack, "t1": t1})
    return in_maps


def kernel(input, number_mask, n, concat, **_):
    assert int(n) == NSEG and int(concat) == 1
    nc = _get_nc()
    in_maps = make_in_maps(input, number_mask)
    res = bass_utils.run_bass_kernel_spmd(nc, in_maps, core_ids=list(range(NCORES)))
    return np.concatenate([res.results[c]["out"] for c in range(NCORES)], axis=0)


# revision 18
# speedup vs baseline: 1.9407x; 1.1722x over previous
"""Trainium2 Bass kernel for nn_AwareDecoder segment first/last gather.

Problem: input [16, 2048, 1024] f32, number_mask [16, 2048] int64 with ids in
[0, 512]. For each segment id i in [0, 512): find first/last row-major token
position with that id, gather those rows of the flattened input, concat ->
out [512, 2048] f32.

Strategy (8 NeuronCores, segment-sharded, no collectives). Core c owns
segments [64c, 64c+64). Token t = 128*g + q with q on partitions, chunk
g in [0, 256); g = 8*cls + gl (class cls in [0, 32), offset gl in [0, 8)).

  1. DVE: one double-broadcast is_equal builds the 0/1 mask
     [128q, 256g, 64s] bf16 (2x packed mode), in 4 g-chunks.
  2. PE: one 512-col matmul per class cls against a fixed stationary
     W[q, 2r+j] = 2^(q-64) (j=0, last) / 2^(63-q) (j=1, first), PSUM-
     accumulated over 16 classes per half. A sum of distinct powers of 2
     per (s, g): the f32 exponent IS the winning q within the chunk.
  3. Act drains PSUM -> SBUF bf16; DMA-transposes (XBAR) put (gl parity,
     seg) on partitions with zero engine time.
  4. DVE int16 decode (4x/2x modes): exponent extract via bitcast>>7,
     add a position table, kill empty chunks, max-reduce, combine halves
     and parities -> first/last token index per segment.
  5. One hardware indirect DMA gathers the 128 rows (4KB each) from HBM;
     direct DMAs write the [64, 2048] output slice.
Host concatenates the 8 slices.
"""
import numpy as np
import ml_dtypes

import concourse.bass as bass
import concourse.tile as tile
from concourse import bacc, mybir
from concourse import bass_utils
from concourse.masks import make_identity

P = 128            # partitions / tokens per chunk
L = 32768          # B*S tokens
H = 1024           # hidden
NSEG = 512         # segments
NCORES = 8
SEG = NSEG // NCORES                     # 64 segments per core
NG = L // P                              # 256 chunks
GL = 8                                   # chunk offsets per class
NCLS = NG // GL                          # 32 classes
F32 = mybir.dt.float32
F16 = mybir.dt.float16
BF16 = mybir.dt.bfloat16
I16 = mybir.dt.int16
I32 = mybir.dt.int32

NCHUNK = 4                 # mask/matmul pipeline chunks
GPC = NG // NCHUNK         # 64 g per chunk
CPC = NCLS // NCHUNK       # 8 classes per chunk


def build_nc():
    nc = bacc.Bacc("TRN2", target_bir_lowering=False, debug=False)

    x = nc.dram_tensor("x", [L, H], F32, kind="ExternalInput")
    # fpack[q, :SEG] = segp; then 4 chunks of duplicated id pairs (f16)
    fpack_in = nc.dram_tensor("fpack", [P, SEG + 2 * NG], F16,
                              kind="ExternalInput")
    # 32 stationaries W_c[q, 2r'+j], c-major
    w_in = nc.dram_tensor("wstack", [P, NCLS * 32], BF16, kind="ExternalInput")
    # decode table t1[s, 256h + 32gl + 2r' + j]
    t1_in = nc.dram_tensor("t1", [SEG, 512], I16, kind="ExternalInput")
    out = nc.dram_tensor("out", [SEG, 2 * H], F32, kind="ExternalOutput")

    with tile.TileContext(nc) as tc:
        with tc.tile_pool(name="sb", bufs=1) as sb, \
             tc.tile_pool(name="big", bufs=1) as big, \
             tc.tile_pool(name="ps", bufs=1, space="PSUM") as ps:

            fpack_t = sb.tile([P, SEG + 2 * NG], F16)
            w_t = sb.tile([P, NCLS * 32], BF16)
            for k in range(NCHUNK):
                eng = nc.sync if k % 2 == 0 else nc.scalar
                lo = SEG + 128 * k if k else 0
                eng.dma_start(fpack_t[:, lo:SEG + 128 * (k + 1)],
                              fpack_in.ap()[:, lo:SEG + 128 * (k + 1)])
                eng.dma_start(w_t[:, 256 * k:256 * (k + 1)],
                              w_in.ap()[:, 256 * k:256 * (k + 1)])
            segp_t = fpack_t[:, 0:SEG]
            ids_t = fpack_t[:, SEG:]
            t1_t = sb.tile([SEG, 512], I16)
            nc.gpsimd.dma_start(t1_t[:], t1_in.ap())
            ident = sb.tile([32, 32], BF16)
            make_identity(nc, ident[:])

            # ---- mask[q, g, s] = (id[q, g] == 64c + s), bf16 0/1 ----
            mask = big.tile([P, NG, SEG], BF16)
            ps_mm = ps.tile([2 * NCLS // 2, 512], F32)      # [32, 512] x2 halves
            ps_mm2 = ps.tile([32, 512], F32)
            drained0 = sb.tile([32, 512], BF16)
            drained1 = sb.tile([32, 512], BF16)
            drained = [drained0, drained1]
            tps = ps.tile([SEG, 2, 8, 32], BF16)
            psh = [ps_mm, ps_mm2]

            for k in range(NCHUNK):
                g0 = k * GPC
                nc.vector.tensor_tensor(
                    out=mask[:, g0:g0 + GPC, :]
                        .rearrange("p g (a b) -> p g a b", b=2),
                    in0=ids_t[:, 2 * g0:2 * (g0 + GPC)]
                        .rearrange("p (g e) -> p g e", e=2).unsqueeze(2)
                        .broadcast_to([P, GPC, SEG // 2, 2]),
                    in1=segp_t.rearrange("p (a b) -> p a b", b=2)
                        .unsqueeze(1).broadcast_to([P, GPC, SEG // 2, 2]),
                    op=mybir.AluOpType.is_equal)
                for cc in range(CPC):
                    c = k * CPC + cc
                    h, r = c // 16, c % 16
                    nc.tensor.matmul(
                        out=psh[h][:, :],
                        lhsT=w_t[:, 32 * c:32 * (c + 1)],
                        rhs=mask[:, GL * c:GL * (c + 1), :],
                        start=(r == 0), stop=(r == 15))
                if k % 2 == 1:                    # half h = k//2 complete
                    h = k // 2
                    nc.scalar.activation(drained[h][:], psh[h][:],
                                         mybir.ActivationFunctionType.Copy)
            for h in range(2):
                for gl in range(GL):
                    nc.tensor.transpose(
                        out=tps[:, h, gl, :],
                        in_=drained[h][:, 64 * gl:64 * (gl + 1)],
                        identity=ident[:])

            # ---- int16 decode ----
            red = sb.tile([SEG, 2, 2], I16)
            for h in range(2):
                srcap = tps[:, h, :, :].rearrange("p q m -> p (q m)")
                ebs = sb.tile([SEG, 256], I16, tag="ebs")
                nc.vector.tensor_scalar(ebs[:], srcap.bitcast(I16), 7, None,
                                        op0=mybir.AluOpType.logical_shift_right)
                ind = sb.tile([SEG, 256], I16, tag="ind")
                nc.vector.tensor_scalar(ind[:], ebs[:], 0, None,
                                        op0=mybir.AluOpType.is_gt)
                val = sb.tile([SEG, 256], I16, tag="val")
                nc.vector.tensor_tensor(out=val[:], in0=ebs[:],
                                        in1=t1_t[:, 256 * h:256 * (h + 1)],
                                        op=mybir.AluOpType.add)
                nc.vector.tensor_tensor(out=val[:], in0=val[:], in1=ind[:],
                                        op=mybir.AluOpType.mult)
                nc.vector.tensor_reduce(red[:, h, :],
                                        val[:].rearrange("p (u j) -> p j u", j=2),
                                        axis=mybir.AxisListType.X,
                                        op=mybir.AluOpType.max)
            fin = sb.tile([SEG, 2], I16)
            nc.vector.tensor_tensor(out=fin[:], in0=red[:, 0, :],
                                    in1=red[:, 1, :], op=mybir.AluOpType.max)
            idx = sb.tile([SEG, 2], I32)
            nc.vector.tensor_scalar(idx[:, 0:1], fin[:, 1:2], -1, 32767,
                                    op0=mybir.AluOpType.mult,
                                    op1=mybir.AluOpType.add)
            nc.vector.tensor_scalar_add(idx[:, 1:2], fin[:, 0:1], 0)

            # ---- gather rows + write out ----
            rows_f = big.tile([SEG, H], F32)
            rows_l = big.tile([SEG, H], F32)
            nc.gpsimd.indirect_dma_start(
                out=rows_f[:], out_offset=None, in_=x.ap(),
                in_offset=bass.IndirectOffsetOnAxis(ap=idx[:, 0:1], axis=0),
                bounds_check=L - 1, oob_is_err=False)
            nc.gpsimd.indirect_dma_start(
                out=rows_l[:], out_offset=None, in_=x.ap(),
                in_offset=bass.IndirectOffsetOnAxis(ap=idx[:, 1:2], axis=0),
                bounds_check=L - 1, oob_is_err=False)
            nc.scalar.dma_start(out.ap()[:, 0:H], rows_f[:])
            nc.sync.dma_start(out.ap()[:, H:2 * H], rows_l[:])

    nc.compile()
    return nc


_NC = None


def _get_nc():
    global _NC
    if _NC is None:
        _NC = build_nc()
    return _NC


def make_in_maps(input, number_mask):
    x = np.ascontiguousarray(np.asarray(input), dtype=np.float32).reshape(L, H)
    nm = np.asarray(number_mask).reshape(L).astype(np.int64)
    ids = np.ascontiguousarray(nm.reshape(NG, P).T).astype(np.float16)
    ids2 = np.repeat(ids, 2, axis=1)

    q = np.arange(P, dtype=np.float64)
    wstack = np.zeros((P, NCLS, 32), dtype=np.float64)
    for c in range(NCLS):
        r = c % 16
        wstack[:, c, 2 * r] = np.exp2(q - 64.0)
        wstack[:, c, 2 * r + 1] = np.exp2(63.0 - q)
    wstack = wstack.reshape(P, NCLS * 32).astype(ml_dtypes.bfloat16)

    t1 = np.empty((2, 8, 16, 2), dtype=np.int16)
    for h in range(2):
        for gl in range(8):
            for r in range(16):
                g = 8 * (16 * h + r) + gl
                t1[h, gl, r, 0] = np.int16(128 * g - 63)
                t1[h, gl, r, 1] = np.int16(32577 - 128 * g)
    t1 = np.tile(t1.reshape(1, 512), (SEG, 1))

    in_maps = []
    for c in range(NCORES):
        segp = np.tile(np.arange(SEG, dtype=np.float16) + c * SEG, (P, 1))
        in_maps.append({"x": x,
                        "fpack": np.concatenate([segp, ids2], axis=1),
                        "wstack": wstack, "t1": t1})
    return in_maps


def kernel(input, number_mask, n, concat, **_):
    assert int(n) == NSEG and int(concat) == 1
    nc = _get_nc()
    in_maps = make_in_maps(input, number_mask)
    res = bass_utils.run_bass_kernel_spmd(nc, in_maps, core_ids=list(range(NCORES)))
    return np.concatenate([res.results[c]["out"] for c in range(NCORES)], axis=0)
